# revision 1
# baseline (speedup 1.0000x reference)
"""AgreementRouting (CapsNet dynamic routing) Trainium2 kernel.

Data-parallel over batch B=128 across 8 cores (B_local=16 per core).

Per core, u lives in SBUF as fp16 in an "H layout":
  partition p = b_loc*16 + d   (b_loc in [0,8), d in [0,16))
  free       = (j in [0,10), h in [0,2), i in [0,1152))
local batch index beta = h*8 + b_loc.

Key structural idea: the routing iteration keeps the *unnormalized* s
vector as the PE stationary (s_fat, block-diagonal [128,80] per (j,h));
the squash scale f = sq/((1+sq)sqrt(sq+EPS)) is folded into the logit
update as a per-partition scalar on the [80]-partition side:
    bb += f[b,j] * (sum_d s[b,j,d] * u[(b,d), i])
so the squash never sits on the serial chain, and next-iteration W1
matmuls are gated only by the previous iteration's weighted-sum columns.

Pipeline per iteration (i-chunked at 512 for PSUM-bank granularity):
  W1: PE accumulating matmuls a_c = sum_j s_fat[j].T @ u16[j]   (fp16)
  bb += f80 * a_c                   (DVE scalar_tensor_tensor, in-place)
  e = exp(bb)                       (ACT, shared Exp/Ln table)
  Z-replicated = zselrep.T @ e      (PE, f32, [80,*] pre-replicated)
  lzr = ln(Z)                       (ACT)
  tm = bb - lzr                     (DVE)
  c16 = exp(tm)                     (ACT, fp16)
  W3: c_rep = sel_j.T @ c16         (PE fp16 selector expand to (b,d))
  W4: s_col = sum_i u16*c_rep       (DVE STT fused accumulate; a few j
                                     routed via ACT-copy + GpSimd STT)
  s_fat/f80 rebuilt incrementally as s columns complete.

Numerics vs the fp32 oracle: absmax/scale ~ 5e-4 (verified in numpy,
CoreSim, and on hardware).
"""

import os
import sys

import numpy as np

for _p in ("/opt/trn_rl_repo", "/opt/trn_rl_repo/concourse"):
    if _p not in sys.path and os.path.isdir(_p):
        sys.path.insert(0, _p)

B, IC, OC, D = 128, 1152, 10, 16
NCORES = 8
BL = B // NCORES          # 16 local batch
H = 2                     # halves of local batch
BLOC = BL // H            # 8
NI = IC                   # 1152
EPS = 1e-8
NITER = 3
CHUNKS = [(0, 512), (512, 1024), (1024, 1152)]
NGP_J = int(os.environ.get("K_NGP_J", "0"))   # per-h-block W4 ops -> GpSimd
NMIX_J = int(os.environ.get("K_NMIX_J", "0"))  # per-h-block W4 ops -> ACT-assist
NGP_S0 = int(os.environ.get("K_NGP_S0", "4"))  # of 20 s0 ops -> GpSimd
NGP_BB = int(os.environ.get("K_NGP_BB", "0"))  # of 6 bb-update chunks -> GpSimd
TIME_REPS = int(os.environ.get("K_TIME_REPS", "1"))  # whole-program reps

_PROG_CACHE = {}


def _build_consts():
    """Host-side constant selector matrices."""
    # sel16[(j2,b), j, (bl,d)] = 1 if j2==j and b==bl  -> [80, 10, 128] fp16
    sel = np.zeros((OC * BLOC, OC, BLOC * D), np.float16)
    for j in range(OC):
        for b in range(BLOC):
            sel[j * BLOC + b, j, b * D:(b + 1) * D] = 1.0
    # zselrep[(j,b), (j2,b2)] = 1 if b==b2               -> [80, 80] f32
    zselrep = np.zeros((OC * BLOC, OC * BLOC), np.float32)
    for j in range(OC):
        for b in range(BLOC):
            for j2 in range(OC):
                zselrep[j * BLOC + b, j2 * BLOC + b] = 1.0
    # base8[(b,d), b2] = 1 if b==b2                      -> [128, 8] f32
    base8 = np.zeros((BLOC * D, BLOC), np.float32)
    for b in range(BLOC):
        base8[b * D:(b + 1) * D, b] = 1.0
    dsel = base8
    drep = np.ascontiguousarray(base8.T)
    return dict(sel16=sel, zselrep=zselrep, base8=base8,
                dsel=dsel, drep=drep)


def _build_program(general_b):
    import concourse.bacc as bacc
    import concourse.mybir as mybir
    import concourse.tile as tile

    dt = mybir.dt
    AF = mybir.ActivationFunctionType
    ALU = mybir.AluOpType

    # Force a single shared ACT table (Exp+Ln+Copy+Identity in one set) so
    # the table-load pass emits one load instead of thrashing per func.
    from concourse import hw_specs as _hws
    _orig_tabs = _hws.get_activation_tables
    _keep = "natural_log_exp_and_others"

    def _patched_tabs(arch, __orig=_orig_tabs, __keep=_keep):
        tabs = __orig(arch)
        return {n: (s if n == __keep else set()) for n, s in tabs.items()}

    bacc.get_activation_tables = _patched_tabs

    nc = bacc.Bacc("TRN2", target_bir_lowering=False, debug=False)

    # ---- DRAM I/O ----
    u16_d = nc.dram_tensor("u16", [128, OC, H, NI], dt.float16,
                           kind="ExternalInput").ap()
    sel_d = nc.dram_tensor("sel16", [OC * BLOC, OC, BLOC * D], dt.float16,
                           kind="ExternalInput").ap()
    zselrep_d = nc.dram_tensor("zselrep", [OC * BLOC, OC * BLOC], dt.float32,
                               kind="ExternalInput").ap()
    base8_d = nc.dram_tensor("base8", [BLOC * D, BLOC], dt.float32,
                             kind="ExternalInput").ap()
    dsel_d = nc.dram_tensor("dsel", [BLOC * D, BLOC], dt.float32,
                            kind="ExternalInput").ap()
    drep_d = nc.dram_tensor("drep", [BLOC, BLOC * D], dt.float32,
                            kind="ExternalInput").ap()
    if general_b:
        c0_d = nc.dram_tensor("c0rep", [128, OC, NI], dt.float16,
                              kind="ExternalInput").ap()
        bb0_d = nc.dram_tensor("bb0", [OC * BLOC, NI], dt.float32,
                               kind="ExternalInput").ap()
    out_d = nc.dram_tensor("vout", [128, 2 * OC], dt.float32,
                           kind="ExternalOutput").ap()

    # ---- static SBUF ----
    def sb(name, shape, dtype):
        return nc.alloc_sbuf_tensor(name, list(shape), dtype).ap()

    u16 = sb("u16_sb", [128, OC * H * NI], dt.float16)       # 46KB/part
    sel_sb = sb("sel_sb", [OC * BLOC, OC * BLOC * D], dt.float16)
    zselrep_sb = sb("zselrep_sb", [OC * BLOC, OC * BLOC], dt.float32)
    base8_sb = sb("base8_sb", [BLOC * D, BLOC], dt.float32)
    dsel_sb = sb("dsel_sb", [BLOC * D, BLOC], dt.float32)
    drep_sb = sb("drep_sb", [BLOC, BLOC * D], dt.float32)
    bb = [sb(f"bbsb{h}", [OC * BLOC, NI], dt.float32) for h in range(H)]
    sfat = [[sb(f"sfat{j}_{h}", [128, OC * BLOC], dt.float16)
             for h in range(H)] for j in range(OC)]
    f80 = [sb(f"f80_{h}", [OC * BLOC, 1], dt.float32) for h in range(H)]
    bias80 = sb("bias80", [OC * BLOC, 1], dt.float32)
    s_sb = sb("s_sb", [128, H * OC], dt.float32)
    # mini-squash scratch (per h)
    ssqh = [sb(f"ssqh{h}", [128, OC], dt.float32) for h in range(H)]
    sepsh = [sb(f"sepsh{h}", [BLOC, OC], dt.float32) for h in range(H)]
    lnxh = [sb(f"lnxh{h}", [BLOC, OC], dt.float32) for h in range(H)]
    rh = [sb(f"rh{h}", [BLOC, OC], dt.float32) for h in range(H)]
    t1h = [sb(f"t1h{h}", [BLOC, OC], dt.float32) for h in range(H)]
    denh = [sb(f"denh{h}", [BLOC, OC], dt.float32) for h in range(H)]
    rech = [sb(f"rech{h}", [BLOC, OC], dt.float32) for h in range(H)]
    fh = [sb(f"fh{h}", [BLOC, OC], dt.float32) for h in range(H)]
    # final squash scratch
    ssq_sb = sb("ssq_sb", [128, H * OC], dt.float32)
    seps_sb = sb("seps_sb", [BLOC, H * OC], dt.float32)
    lnx_sb = sb("lnx_sb", [BLOC, H * OC], dt.float32)
    r_sb = sb("r_sb", [BLOC, H * OC], dt.float32)
    t1_sb = sb("t1_sb", [BLOC, H * OC], dt.float32)
    den_sb = sb("den_sb", [BLOC, H * OC], dt.float32)
    rec_sb = sb("rec_sb", [BLOC, H * OC], dt.float32)
    f_sb = sb("f_sb", [BLOC, H * OC], dt.float32)
    v_sb = sb("v_sb", [128, H * OC], dt.float32)

    def uview(j, h):
        off = (j * H + h) * NI
        return u16[:, off:off + NI]

    def selview(j):
        return sel_sb[:, j * BLOC * D:(j + 1) * BLOC * D]

    with tile.TileContext(nc) as tc:
        from contextlib import ExitStack
        with ExitStack() as ctx:
            psA = ctx.enter_context(
                tc.tile_pool(name="psA", bufs=int(os.environ.get("K_CRBUFS", "2")), space="PSUM"))
            psB = ctx.enter_context(
                tc.tile_pool(name="psB", bufs=2, space="PSUM"))
            psZ = psB
            sc = ctx.enter_context(tc.tile_pool(name="sc", bufs=int(os.environ.get("K_SCBUFS", "6"))))
            ec = ctx.enter_context(tc.tile_pool(name="ec", bufs=int(os.environ.get("K_ECBUFS", "8"))))

            for _rep in range(TIME_REPS):
                # ---- loads ----
                for j in range(OC):
                    nc.sync.dma_start(
                        u16[:, j * H * NI:(j + 1) * H * NI], u16_d[:, j, :, :])
                nc.sync.dma_start(sel_sb[:], sel_d.rearrange("p j m -> p (j m)"))
                nc.sync.dma_start(zselrep_sb[:], zselrep_d)
                nc.sync.dma_start(base8_sb[:], base8_d)
                nc.sync.dma_start(dsel_sb[:], dsel_d)
                nc.sync.dma_start(drep_sb[:], drep_d)

                # ---- init bb and sfat ----
                for h in range(H):
                    if general_b:
                        nc.sync.dma_start(bb[h][:], bb0_d)
                    else:
                        nc.gpsimd.memset(bb[h][:], 0.0)
                for j in range(OC):
                    for h in range(H):
                        nc.gpsimd.memset(sfat[j][h][:], 0.0)

                def build_sfat(j, h):
                    col = 2 * j + h
                    nc.vector.tensor_scalar(
                        out=sfat[j][h][:, j * BLOC:(j + 1) * BLOC],
                        in0=base8_sb[:],
                        scalar1=s_sb[:, col:col + 1],
                        scalar2=None,
                        op0=ALU.mult)

                def mini_squash(h):
                    """f80[h] <- squash scale from s_sb columns of parity h."""
                    s_h = s_sb[:, h::2]  # [128, OC] strided view
                    nc.vector.tensor_tensor(ssqh[h][:], s_h, s_h, op=ALU.mult)
                    sq_ps = psB.tile([BLOC, OC], dt.float32, tag="bank",
                                     name="sq_ps")
                    nc.tensor.matmul(sq_ps[:], dsel_sb[:], ssqh[h][:],
                                     start=True, stop=True)
                    nc.vector.tensor_scalar_add(sepsh[h][:], sq_ps[:], EPS)
                    nc.scalar.activation(lnxh[h][:], sepsh[h][:], AF.Ln)
                    nc.scalar.activation(rh[h][:], lnxh[h][:], AF.Exp, scale=0.5)
                    nc.vector.tensor_scalar_add(t1h[h][:], sq_ps[:], 1.0)
                    nc.vector.tensor_tensor(denh[h][:], t1h[h][:], rh[h][:],
                                            op=ALU.mult)
                    nc.vector.reciprocal(rech[h][:], denh[h][:])
                    nc.vector.tensor_tensor(fh[h][:], sq_ps[:],
                                            rech[h][:], op=ALU.mult)
                    for j in range(OC):
                        nc.sync.dma_start(
                            f80[h][j * BLOC:(j + 1) * BLOC, 0:1],
                            fh[h][:, j:j + 1])

                # ---- init s0 ----
                if general_b:
                    c0_sb = sc.tile([128, OC * NI], dt.float16, tag="c0",
                                    name="c0_sb", bufs=1)
                    nc.sync.dma_start(c0_sb[:], c0_d)
                    for j in range(OC):
                        for h in range(H):
                            col = 2 * j + h
                            scr = sc.tile([128, NI], dt.float16, tag="scr",
                                          name="scr")
                            nc.vector.scalar_tensor_tensor(
                                out=scr[:], in0=uview(j, h), scalar=1.0,
                                in1=c0_sb[:, j * NI:(j + 1) * NI],
                                op0=ALU.mult, op1=ALU.mult,
                                accum_out=s_sb[:, col:col + 1])
                else:
                    # s0 = 0.1 * sum_i u  via tensor_scalar+accum (DVE 4x
                    # perf mode: all-fp16 SBUF). Split a few ops to GpSimd.
                    for k, (j, h) in enumerate(
                            [(j, h) for j in range(OC) for h in range(H)]):
                        col = 2 * j + h
                        scr = sc.tile([128, NI], dt.float16, tag="scr",
                                      name="scr")
                        nc.vector.tensor_scalar(
                            out=scr[:], in0=uview(j, h), scalar1=1.0 / OC,
                            scalar2=0.0, op0=ALU.mult, op1=ALU.add,
                            accum_out=s_sb[:, col:col + 1])
                for j in range(OC):
                    for h in range(H):
                        build_sfat(j, h)
                for h in range(H):
                    mini_squash(h)

                # ---- routing iterations, software-pipelined by h ----
                # emit order: mid(0,0) mid(0,1) w4(0,0) mid(1,0) w4(0,1)
                # mid(1,1) w4(1,0) mid(2,0) w4(1,1) mid(2,1) w4(2,0) w4(2,1)
                # so every engine has an independent ready block between
                # W4 blocks (mid(k+1,h) depends only on w4(k,h)).
                c16 = {}

                def emit_mid(it, h):
                    """W1 + bb update + softmax chain -> c16[(it, h)]."""
                    a_t = {}
                    for ci, (c0, c1) in enumerate(CHUNKS):
                        a_t[ci] = psB.tile([OC * BLOC, 512], dt.float32,
                                           tag="bank", name="a_c")
                    # j-outer so each sfat stationary loads once
                    for j in range(OC):
                        for ci, (c0, c1) in enumerate(CHUNKS):
                            nc.tensor.matmul(
                                a_t[ci][:, :c1 - c0], sfat[j][h],
                                uview(j, h)[:, c0:c1],
                                start=(j == 0), stop=(j == OC - 1))
                    for ci, (c0, c1) in enumerate(CHUNKS):
                        # always DVE: GPSIMD cannot read PSUM (a_t)
                        nc.vector.scalar_tensor_tensor(
                            out=bb[h][:, c0:c1], in0=a_t[ci][:, :c1 - c0],
                            scalar=f80[h][:, 0:1], in1=bb[h][:, c0:c1],
                            op0=ALU.mult, op1=ALU.add)
                    e_t, z_t, lzr_t, rz_t = {}, {}, {}, {}
                    for ci, (c0, c1) in enumerate(CHUNKS):
                        e_c = ec.tile([OC * BLOC, 512], dt.float32,
                                      tag="e", name="e_c")
                        nc.scalar.activation(e_c[:, :c1 - c0], bb[h][:, c0:c1],
                                             AF.Exp)
                        e_t[ci] = e_c
                    for ci, (c0, c1) in enumerate(CHUNKS):
                        z_c = psZ.tile([OC * BLOC, 512], dt.float32,
                                       tag="bank", name="z_c")
                        nc.tensor.matmul(z_c[:, :c1 - c0], zselrep_sb[:],
                                         e_t[ci][:, :c1 - c0],
                                         start=True, stop=True)
                        z_t[ci] = z_c
                    for ci, (c0, c1) in enumerate(CHUNKS):
                        lzr_c = ec.tile([OC * BLOC, 512], dt.float32,
                                        tag="lzr", name="lzr_c")
                        nc.scalar.activation(lzr_c[:, :c1 - c0],
                                             z_t[ci][:, :c1 - c0], AF.Ln)
                        lzr_t[ci] = lzr_c
                    for ci, (c0, c1) in enumerate(CHUNKS):
                        tm_c = ec.tile([OC * BLOC, 512], dt.float32,
                                       tag="rz", name="tm_c")
                        nc.vector.tensor_tensor(tm_c[:, :c1 - c0],
                                                bb[h][:, c0:c1],
                                                lzr_t[ci][:, :c1 - c0],
                                                op=ALU.subtract)
                        rz_t[ci] = tm_c
                    c16[(it, h)] = ec.tile([OC * BLOC, NI], dt.float16,
                                           tag="c16", name="c16")
                    for ci, (c0, c1) in enumerate(CHUNKS):
                        nc.scalar.activation(c16[(it, h)][:, c0:c1],
                                             rz_t[ci][:, :c1 - c0], AF.Exp)

                def emit_w4(it, h):
                    last = it == NITER - 1
                    ngp = 0
                    nmx = 0
                    for j in range(OC):
                        col = 2 * j + h
                        cr_ps = psA.tile([128, NI], dt.float32, tag="big",
                                         name="cr_ps")
                        for (c0, c1) in CHUNKS:
                            nc.tensor.matmul(cr_ps[:, c0:c1], selview(j),
                                             c16[(it, h)][:, c0:c1],
                                             start=True, stop=True)
                        if j % 4 == 2 and nmx < NMIX_J:
                            # ACT copy psum->fp16 (frees cr early), DVE
                            # TT at 2x, ACT fused accumulate-reduce
                            nmx += 1
                            crs = sc.tile([128, NI], dt.float16, tag="crs",
                                          name="crs")
                            nc.scalar.copy(crs[:], cr_ps[:])
                            w16 = sc.tile([128, NI], dt.float16,
                                          tag="w16", name="w16")
                            nc.vector.tensor_tensor(w16[:], uview(j, h),
                                                    crs[:], op=ALU.mult)
                            scr = sc.tile([128, NI], dt.float16,
                                          tag="scrg", name="scr")
                            nc.scalar.activation(
                                scr[:], w16[:], AF.Identity,
                                accum_out=s_sb[:, col:col + 1])
                        else:
                            scr = sc.tile([128, NI], dt.float16,
                                          tag="scr", name="scr")
                            nc.vector.scalar_tensor_tensor(
                                out=scr[:], in0=uview(j, h), scalar=1.0,
                                in1=cr_ps[:],
                                op0=ALU.mult, op1=ALU.mult,
                                accum_out=s_sb[:, col:col + 1])
                        if not last:
                            build_sfat(j, h)
                    if not last:
                        mini_squash(h)

                blocks = [("m", 0, 0), ("m", 0, 1)]
                for it in range(NITER):
                    blocks.append(("w", it, 0))
                    if it + 1 < NITER:
                        blocks.append(("m", it + 1, 0))
                    blocks.append(("w", it, 1))
                    if it + 1 < NITER:
                        blocks.append(("m", it + 1, 1))
                for kind, it, h in blocks:
                    if kind == "m":
                        emit_mid(it, h)
                    else:
                        emit_w4(it, h)

                # ---- final squash -> v ----
                nc.vector.tensor_tensor(ssq_sb[:], s_sb[:], s_sb[:], op=ALU.mult)
                sq_ps = psB.tile([BLOC, H * OC], dt.float32, tag="bank",
                                 name="sq_ps")
                nc.tensor.matmul(sq_ps[:], dsel_sb[:], ssq_sb[:],
                                 start=True, stop=True)
                nc.vector.tensor_scalar_add(seps_sb[:], sq_ps[:], EPS)
                nc.scalar.activation(lnx_sb[:], seps_sb[:], AF.Ln)
                nc.scalar.activation(r_sb[:], lnx_sb[:], AF.Exp, scale=0.5)
                nc.vector.tensor_scalar_add(t1_sb[:], sq_ps[:], 1.0)
                nc.vector.tensor_tensor(den_sb[:], t1_sb[:], r_sb[:],
                                        op=ALU.mult)
                nc.vector.reciprocal(rec_sb[:], den_sb[:])
                nc.vector.tensor_tensor(f_sb[:], sq_ps[:], rec_sb[:],
                                        op=ALU.mult)
                f_ps = psB.tile([128, H * OC], dt.float32, tag="bank",
                                name="f_ps")
                nc.tensor.matmul(f_ps[:], drep_sb[:], f_sb[:],
                                 start=True, stop=True)
                nc.vector.tensor_tensor(v_sb[:], s_sb[:], f_ps[:], op=ALU.mult)
                nc.sync.dma_start(out_d, v_sb[:])

    nc.compile()
    return nc


def _get_program(general_b):
    key = (bool(general_b), NGP_J)
    if key not in _PROG_CACHE:
        _PROG_CACHE[key] = _build_program(key[0])
    return _PROG_CACHE[key]


def _prep_inputs(u_predict, b):
    """Host-side shard + layout transform. Returns (in_maps, general_b)."""
    general_b = bool(np.any(b != 0.0))
    consts = _build_consts()
    u16 = u_predict.astype(np.float16)
    u6 = u16.reshape(NCORES, H, BLOC, IC, OC, D)
    ut = np.ascontiguousarray(u6.transpose(0, 2, 5, 4, 1, 3))
    ut = ut.reshape(NCORES, 128, OC, H, NI)

    extra = {}
    if general_b:
        bm = b.astype(np.float64)
        e = np.exp(bm - bm.max(axis=1, keepdims=True))
        c0 = (e / e.sum(axis=1, keepdims=True)).astype(np.float16)  # [IC, OC]
        c0rep = np.ascontiguousarray(
            np.broadcast_to(c0.T[None, :, :], (128, OC, NI))).astype(
                np.float16)
        bt = b.astype(np.float32).T  # [OC, NI]
        bb0 = np.ascontiguousarray(
            np.repeat(bt[:, None, :], BLOC, axis=1)).reshape(OC * BLOC, NI)
        extra = {"c0rep": c0rep, "bb0": bb0}

    in_maps = []
    for c in range(NCORES):
        m = {"u16": ut[c],
             "sel16": consts["sel16"], "zselrep": consts["zselrep"],
             "base8": consts["base8"],
             "dsel": consts["dsel"], "drep": consts["drep"]}
        m.update(extra)
        in_maps.append(m)
    return in_maps, general_b


def _gather_output(results):
    out = np.empty((B, OC, D), np.float32)
    for c in range(NCORES):
        v = results[c]["vout"]                  # [p=(bl,d), col=(j*2+h)]
        v4 = v.reshape(BLOC, D, OC, H)          # bl, d, j, h
        out[c * BL:(c + 1) * BL] = v4.transpose(3, 0, 2, 1).reshape(
            BL, OC, D)
    return out


def kernel(u_predict, b=None, **kw):
    u_predict = np.asarray(u_predict, dtype=np.float32)
    if b is None:
        b = np.zeros((IC, OC), np.float32)
    b = np.asarray(b, dtype=np.float32)
    in_maps, general_b = _prep_inputs(u_predict, b)
    nc = _get_program(general_b)

    if os.environ.get("BASS_KERNEL_SIM"):
        from concourse.bass_interp import CoreSim
        sim = CoreSim(nc, trace=False)
        for name, arr in in_maps[0].items():
            sim.tensor(name)[:] = arr
        sim.simulate(check_with_hw=False)
        v0 = np.array(sim.tensor("vout"))
        out = np.empty((B, OC, D), np.float32)
        v4 = v0.reshape(BLOC, D, OC, H)
        out[:BL] = v4.transpose(3, 0, 2, 1).reshape(BL, OC, D)
        return out  # NOTE: only core 0 valid in sim mode

    from concourse import bass_utils
    trace = bool(os.environ.get("BASS_KERNEL_TRACE"))
    res = bass_utils.run_bass_kernel_spmd(
        nc, in_maps, core_ids=list(range(NCORES)), trace=trace)
    kernel.last_results = res
    return _gather_output(res.results)



# revision 4
# speedup vs baseline: 1.1297x; 1.1297x over previous
"""AgreementRouting (CapsNet dynamic routing) Trainium2 kernel, v2.

Data-parallel over batch B=128 across 8 cores (16 per core, split into
two halves h of 8). Per core, TWO SBUF copies of u (fp16):
  u16 [128=(b,d), (j,h,i)]  -- W1 moving operand (d-contraction)
  u_i [128=i%128, (h,t,j,(b,d))] -- W4 stationary tiles (i-contraction)

Routing state is kept in exp domain: M = exp(b0 + sum_t f_t a_t) (bf16,
unnormalized, range-safe), where a = s^T u on PE and the squash scale f
is applied inside the ACT exp as a per-partition scale. c = M / Z is
formed in i-partition layout (PE transposes + one DVE divide per tile)
and consumed by PE W4 matmuls with u stationary:
  s[(b,d), (j,b')] = sum_i u[i,(b,d)] c[b',i,j]   (diag b'==b valid)
The squash scale chain runs on [1,80] rows and returns to [80,1]
per-partition form via a PE transpose. No DMA inside the loop.
"""

import os
import sys

import numpy as np

for _p in ("/opt/trn_rl_repo", "/opt/trn_rl_repo/concourse"):
    if _p not in sys.path and os.path.isdir(_p):
        sys.path.insert(0, _p)

B, IC, OC, D = 128, 1152, 10, 16
NCORES = 8
BL = B // NCORES          # 16 local batch
H = 2                     # halves of local batch
BLOC = BL // H            # 8
NI = IC                   # 1152
NT = IC // 128            # 9 i-tiles
EPS = 1e-8
NITER = 3
CHUNKS = [(0, 512), (512, 1024), (1024, 1152)]

_PROG_CACHE = {}
BLOCKS = []


def _build_consts():
    mask = np.zeros((128, OC * BLOC), np.float16)
    for b in range(BLOC):
        for d in range(D):
            for j in range(OC):
                mask[b * D + d, j * BLOC + b] = 1.0
    import ml_dtypes
    eye80 = np.eye(80, dtype=ml_dtypes.bfloat16)
    eyef32 = np.eye(8, dtype=np.float32)
    onescol = np.ones((128, 1), np.float16)
    ones1 = np.ones((1, 128), np.float32)
    return dict(mask16=mask, eye80=eye80, eyef32=eyef32,
                onescol=onescol, ones1=ones1)


def _build_program(general_b):
    import concourse.bacc as bacc
    import concourse.mybir as mybir
    import concourse.tile as tile

    dt = mybir.dt
    AF = mybir.ActivationFunctionType
    ALU = mybir.AluOpType

    # Single shared ACT table (Exp+Ln+Copy+Identity) so the table-load
    # pass emits one load instead of thrashing per func.
    from concourse import hw_specs as _hws
    _orig_tabs = _hws.get_activation_tables
    _keep = "natural_log_exp_and_others"

    def _patched_tabs(arch, __orig=_orig_tabs, __keep=_keep):
        tabs = __orig(arch)
        return {n: (s if n == __keep else set()) for n, s in tabs.items()}

    bacc.get_activation_tables = _patched_tabs

    nc = bacc.Bacc("TRN2", target_bir_lowering=False, debug=False)

    # ---- DRAM I/O ----
    u16_d = nc.dram_tensor("u16", [128, OC * H * NI], dt.float16,
                           kind="ExternalInput").ap()
    ui_d = nc.dram_tensor("ui", [128, H * NT * OC * 128], dt.float16,
                          kind="ExternalInput").ap()
    mask_d = nc.dram_tensor("mask16", [128, OC * BLOC], dt.float16,
                            kind="ExternalInput").ap()
    eye80_d = nc.dram_tensor("eye80", [80, 80], dt.bfloat16,
                             kind="ExternalInput").ap()
    eyef32_d = nc.dram_tensor("eyef32", [8, 8], dt.float32,
                              kind="ExternalInput").ap()
    onescol_d = nc.dram_tensor("onescol", [128, 1], dt.float16,
                               kind="ExternalInput").ap()
    ones1_d = nc.dram_tensor("ones1", [1, 128], dt.float32,
                             kind="ExternalInput").ap()
    if general_b:
        c0i_d = nc.dram_tensor("c0i", [128, NT * OC], dt.float16,
                               kind="ExternalInput").ap()
        c0rep_d = nc.dram_tensor("c0rep80", [OC * BLOC, NI], dt.bfloat16,
                                 kind="ExternalInput").ap()
    out_d = nc.dram_tensor("vout", [128, H * OC * BLOC], dt.float32,
                           kind="ExternalOutput").ap()

    # ---- static SBUF ----
    def sb(name, shape, dtype):
        return nc.alloc_sbuf_tensor(name, list(shape), dtype).ap()

    u16 = sb("u16_sb", [128, OC * H * NI], dt.float16)
    ui = sb("ui_sb", [128, H * NT * OC * 128], dt.float16)
    mask = sb("mask_sb", [128, OC * BLOC], dt.float16)
    eye80 = sb("eye80_sb", [80, 80], dt.bfloat16)
    eyef32 = sb("eyef32_sb", [8, 8], dt.float32)
    onescol = sb("onescol_sb", [128, 1], dt.float16)
    ones1 = sb("ones1_sb", [1, 128], dt.float32)
    lnc0 = sb("lnc0_sb", [80, 1], dt.float32)
    dumw = sb("dumw_sb", [128, 80], dt.float16)
    s0sb = [sb(f"s0sb{h}", [128, OC], dt.float32) for h in range(H)]
    scr0 = sb("scr0_sb", [128, NI], dt.float16)
    Mt = [sb(f"M{h}", [80, NI], dt.bfloat16) for h in range(H)]
    xt = [sb(f"x{h}", [80, NI], dt.bfloat16) for h in range(H)]
    zsum = [sb(f"zsum{h}", [128, NT * BLOC], dt.float32) for h in range(H)]
    zrec = [sb(f"zrec{h}", [128, NT * BLOC], dt.float32) for h in range(H)]
    ci = [sb(f"ci{h}", [128, NT * 80], dt.float16) for h in range(H)]
    s16 = [sb(f"s16_{h}", [128, 80], dt.float16) for h in range(H)]
    s16m = [sb(f"s16m{h}", [128, 80], dt.float16) for h in range(H)]
    ssq = [sb(f"ssq{h}", [128, 80], dt.float16) for h in range(H)]
    sfat_all = [sb(f"sfat_all{h}", [128, OC * 88], dt.float16)
                for h in range(H)]
    seps = [sb(f"seps{h}", [1, 80], dt.float32) for h in range(H)]
    lnx = [sb(f"lnx{h}", [1, 80], dt.float32) for h in range(H)]
    rr = [sb(f"rr{h}", [1, 80], dt.float32) for h in range(H)]
    t1 = [sb(f"t1_{h}", [1, 80], dt.float32) for h in range(H)]
    den = [sb(f"den{h}", [1, 80], dt.float32) for h in range(H)]
    rec = [sb(f"rec{h}", [1, 80], dt.float32) for h in range(H)]
    frow = [sb(f"frow{h}", [1, 80], dt.float32) for h in range(H)]
    f80sb = [sb(f"f80sb{h}", [80, 1], dt.float32) for h in range(H)]
    v16 = sb("v16_sb", [128, H * 80], dt.float32)
    if general_b:
        c0i = sb("c0i_sb", [128, NT * OC], dt.float16)
        c0rep80 = sb("c0rep_sb", [OC * BLOC, NI], dt.bfloat16)

    ps_sA = nc.alloc_psum_tensor("ps_sA", [128, 512], dt.float32).ap()
    ps_sB = nc.alloc_psum_tensor("ps_sB", [128, 512], dt.float32).ap()
    ps_sm = [nc.alloc_psum_tensor(f"ps_sm{h}", [128, 512],
                                  dt.float32).ap() for h in range(H)]
    ps_mi = [nc.alloc_psum_tensor(f"ps_mi{h}", [128, 960],
                                  dt.bfloat16).ap() for h in range(H)]

    def ps_s(h):
        return (ps_sA if h == 0 else ps_sB)[:, 0:80]

    def ps_s0(h):
        return ps_sm[h][:, 0:10]

    def ps_sq(h):
        return ps_sm[h][:1, 10:90]

    def ps_f80(h):
        return ps_sm[h][:80, 90:91]

    def ps_fr(h):
        return ps_sm[h][:, 92:172]

    def ps_mislot(h, t):
        return ps_mi[h][:, 80 * t:80 * t + 80]

    def sfatv(j, h):
        return sfat_all[h][:, j * 88:j * 88 + 80]

    def uview(j, h):
        off = (j * H + h) * NI
        return u16[:, off:off + NI]

    def uiview(h, t, j):
        off = ((h * NT + t) * OC + j) * 128
        return ui[:, off:off + 128]

    LN_C0 = float(np.log(1.0 / OC))

    def mark(label):
        BLOCKS.append((label, nc.next_id()))

    with tile.TileContext(nc) as tc:
        from contextlib import ExitStack
        with ExitStack() as ctx:
            psA = ctx.enter_context(
                tc.tile_pool(name="psA", bufs=2, space="PSUM"))

            # ---- loads ----
            # order: consts, u16-h0, u16-h1, ui-h0, ui-h1 so both s0
            # halves (DVE-accumulated from u16) start early; the W4s of
            # it0 gate on ui arrival instead.
            u16v_s = u16.rearrange("p (j h i) -> p j h i", j=OC, h=H)
            u16v_d = u16_d.rearrange("p (j h i) -> p j h i", j=OC, h=H)
            nc.sync.dma_start(u16v_s[:, :, 0, :], u16v_d[:, :, 0, :])
            nc.sync.dma_start(mask[:], mask_d)
            nc.sync.dma_start(eye80[:], eye80_d)
            nc.sync.dma_start(eyef32[:], eyef32_d)
            nc.sync.dma_start(onescol[:], onescol_d)
            nc.sync.dma_start(ones1[:], ones1_d)
            if general_b:
                nc.sync.dma_start(c0i[:], c0i_d)
                nc.sync.dma_start(c0rep80[:], c0rep_d)
            nc.sync.dma_start(u16v_s[:, :, 1, :], u16v_d[:, :, 1, :])
            for h in range(H):
                nc.sync.dma_start(
                    ui[:, h * NT * OC * 128:(h + 1) * NT * OC * 128],
                    ui_d[:, h * NT * OC * 128:(h + 1) * NT * OC * 128])

            nc.gpsimd.memset(lnc0[:], LN_C0)
            nc.gpsimd.memset(dumw[:], 0.0)
            for h in range(H):
                nc.gpsimd.memset(sfat_all[h][:], 0.0)

            def squashA(h, s_src, bcast, build_sfat):
                """masked s + sfat blocks + ssq + sq matmul."""
                if bcast:
                    nc.vector.tensor_tensor(
                        s16m[h].rearrange("p (j b) -> p j b", j=OC),
                        s_src.unsqueeze(2).broadcast_to([128, OC, BLOC]),
                        mask.rearrange("p (j b) -> p j b", j=OC),
                        op=ALU.mult)
                else:
                    nc.vector.tensor_mul(s16m[h][:], s_src, mask[:])
                if build_sfat:
                    import bass_rust as _br
                    t = sfat_all[h]
                    outap = _br.AP(t.tensor, t.offset,
                                   [list(t.ap)[0], (96, OC), (1, BLOC)])
                    nc.vector.tensor_copy(
                        out=outap,
                        in_=s16m[h].rearrange("p (j b) -> p j b", j=OC))
                nc.vector.tensor_mul(ssq[h][:], s16m[h][:], s16m[h][:])
                sq_ps = ps_sq(h)
                nc.tensor.matmul(sq_ps[:], onescol[:], ssq[h][:],
                                 start=True, stop=True)

            def squashB(h, last):
                """squash scale chain: frow (and f80 unless last)."""
                sq_ps = ps_sq(h)
                nc.vector.tensor_scalar_add(seps[h][:], sq_ps[:], EPS)
                nc.scalar.activation(lnx[h][:], seps[h][:], AF.Ln)
                nc.scalar.activation(rr[h][:], lnx[h][:], AF.Exp, scale=0.5)
                nc.vector.tensor_scalar_add(t1[h][:], seps[h][:], 1.0)
                nc.vector.tensor_mul(den[h][:], t1[h][:], rr[h][:])
                nc.vector.reciprocal(rec[h][:], den[h][:])
                nc.vector.scalar_tensor_tensor(
                    out=frow[h][:], in0=t1[h][:], scalar=-1.0,
                    in1=rec[h][:], op0=ALU.add, op1=ALU.mult)
                if not last:
                    f80 = ps_f80(h)
                    nc.tensor.matmul(f80, frow[h][:], eyef32[:1, :1],
                                     is_transpose=True)
                    nc.vector.tensor_copy(out=f80sb[h][:], in_=f80)

            def squash(h, s_src, bcast, build_sfat, last):
                squashA(h, s_src, bcast, build_sfat)
                squashB(h, last)

            def emit_s0TS(h):
                """accumulate s0; returns the [128, OC] source ap."""
                if general_b:
                    s0_ps = ps_s0(h)
                    for t in range(NT):
                        for j in range(OC):
                            nc.tensor.matmul(
                                s0_ps[:, j:j + 1], uiview(h, t, j),
                                c0i[:, t * OC + j:t * OC + j + 1],
                                start=(t == 0 and j == 0),
                                stop=(t == NT - 1 and j == OC - 1),
                                skip_group_check=True)
                    return s0_ps
                # s0 = (1/OC) * sum_i u -- DVE 4x tensor_scalar+accum
                for j in range(OC):
                    nc.vector.tensor_scalar(
                        out=scr0[:], in0=uview(j, h), scalar1=1.0 / OC,
                        scalar2=0.0, op0=ALU.mult, op1=ALU.add,
                        accum_out=s0sb[h][:, j:j + 1])
                return s0sb[h][:]

            def emit_W1(it, h, cidx):
                c0, c1 = CHUNKS[cidx]
                a = psA.tile([80, 512], dt.float32, tag="a", name="a")
                for j in range(OC):
                    nc.tensor.matmul(a[:, :c1 - c0], sfatv(j, h),
                                     uview(j, h)[:, c0:c1],
                                     start=(j == 0), stop=(j == OC - 1))
                return a

            def emit_expM(it, h, cidx, a):
                first = it == 0
                c0, c1 = CHUNKS[cidx]
                use_bias = first and not general_b
                xdst = Mt[h] if use_bias else xt[h]
                nc.scalar.activation(
                    xdst[:, c0:c1], a[:, :c1 - c0], AF.Exp,
                    bias=(lnc0[:, 0:1] if use_bias else 0.0),
                    scale=f80sb[h][:, 0:1])
                if first and general_b:
                    nc.vector.tensor_mul(Mt[h][:, c0:c1],
                                         c0rep80[:, c0:c1],
                                         xt[h][:, c0:c1])
                elif not first:
                    nc.vector.tensor_mul(Mt[h][:, c0:c1],
                                         Mt[h][:, c0:c1],
                                         xt[h][:, c0:c1])

            def emit_chunk(it, h, cidx):
                a = emit_W1(it, h, cidx)
                emit_expM(it, h, cidx, a)

            def emit_MT(h, t0, t1):
                for t in range(t0, t1):
                    mi = ps_mislot(h, t)
                    nc.tensor.matmul(mi, Mt[h][:, t * 128:(t + 1) * 128],
                                     eye80[:], is_transpose=True,
                                     start=(t == 0), stop=(t == NT - 1),
                                     skip_group_check=True)

            def emit_red(h, t0, t1):
                nt = t1 - t0
                nc.vector.tensor_reduce(
                    out=zsum[h][:, t0 * BLOC:t1 * BLOC],
                    in_=ps_mi[h][:, t0 * 80:t1 * 80].rearrange(
                        "p (t j b) -> p t b j", j=OC, t=nt),
                    axis=mybir.AxisListType.X, op=ALU.add)
                nc.vector.reciprocal(zrec[h][:, t0 * BLOC:t1 * BLOC],
                                     zsum[h][:, t0 * BLOC:t1 * BLOC])
                nc.vector.tensor_tensor(
                    ci[h][:, t0 * 80:t1 * 80].rearrange(
                        "p (t j b) -> p t j b", j=OC, t=nt),
                    ps_mi[h][:, t0 * 80:t1 * 80].rearrange(
                        "p (t j b) -> p t j b", j=OC, t=nt),
                    zrec[h][:, t0 * BLOC:t1 * BLOC].rearrange(
                        "p (t b) -> p t b", t=nt)
                    .unsqueeze(2).broadcast_to([128, nt, OC, BLOC]),
                    op=ALU.mult)

            def emit_A(it, h):
                """W1 + exp + M-update, chunk-pipelined; the first 4
                M-transposes and their z/divide block are interleaved so
                W4 can start right after the last W1 chunk."""
                emit_chunk(it, h, 0)
                emit_chunk(it, h, 1)
                emit_MT(h, 0, 4)
                emit_red(h, 0, 4)
                emit_chunk(it, h, 2)

            def emit_Z(it, h):
                emit_MT(h, 4, NT)
                emit_red(h, 4, NT)

            def emit_W4(it, h):
                sp = ps_s(h)
                for t in range(NT):
                    for j in range(OC):
                        nc.tensor.matmul(
                            sp[:, j * BLOC:(j + 1) * BLOC],
                            uiview(h, t, j),
                            ci[h][:, t * 80 + j * BLOC:
                                  t * 80 + (j + 1) * BLOC],
                            start=(t == 0 and j == 0),
                            stop=(t == NT - 1 and j == OC - 1),
                            skip_group_check=True)
                return sp

            def emit_fin(h):
                frep = ps_fr(h)
                nc.tensor.matmul(frep, ones1[:], frow[h][:],
                                 start=True, stop=True)
                nc.vector.tensor_mul(v16[:, h * 80:(h + 1) * 80],
                                     s16m[h][:], frep)

            # ---- prewarm PE during the u16-h0 load tail ----
            ndum = int(os.environ.get("K_DUMMY", "12"))
            for k in range(ndum):
                dtile = psA.tile([80, 512], dt.float32, tag="a", name="dum")
                nc.tensor.matmul(dtile[:], dumw[:], u16[:, 0:512],
                                 start=True, stop=True)

            # ---- schedule ----
            # h0's data (u16-h0, ui-h0) lands first; h1's W4 of it0 gates
            # on the final ui-h1 DMA, so h0 runs one step ahead through
            # it0/it1 and h1 gets engine priority afterwards.
            def M(label, fn, *a):
                mark(label)
                return fn(*a)

            s0src = M("s0.0", emit_s0TS, 0)
            M("sqAs0.0", squashA, 0, s0src, True, True)
            a00 = M("W1c0.00", emit_W1, 0, 0, 0)
            a01 = M("W1c1.00", emit_W1, 0, 0, 1)
            M("sqBs0.0", squashB, 0, False)
            s0src = M("s0.1", emit_s0TS, 1)
            M("exp0.00", emit_expM, 0, 0, 0, a00)
            M("exp1.00", emit_expM, 0, 0, 1, a01)
            M("MTa.00", emit_MT, 0, 0, 4)
            M("W1c2.00", emit_chunk, 0, 0, 2)
            M("sqAs0.1", squashA, 1, s0src, True, True)
            M("sqBs0.1", squashB, 1, False)
            M("reda.00", emit_red, 0, 0, 4)
            M("Z0.0", emit_Z, 0, 0)
            sp0 = M("W4_0.0", emit_W4, 0, 0)
            M("sqA0.0", squashA, 0, sp0, False, True)
            M("A0.1", emit_A, 0, 1)
            M("sqB0.0", squashB, 0, False)
            M("Z0.1", emit_Z, 0, 1)
            M("A1.0", emit_A, 1, 0)
            sp1 = M("W4_0.1", emit_W4, 0, 1)
            M("sqA0.1", squashA, 1, sp1, False, True)
            M("sqB0.1", squashB, 1, False)
            M("Z1.0", emit_Z, 1, 0)
            sp0 = M("W4_1.0", emit_W4, 1, 0)
            M("sqA1.0", squashA, 0, sp0, False, True)
            M("sqB1.0", squashB, 0, False)
            M("A1.1", emit_A, 1, 1)
            M("Z1.1", emit_Z, 1, 1)
            sp1 = M("W4_1.1", emit_W4, 1, 1)
            M("sqA1.1", squashA, 1, sp1, False, True)
            M("sqB1.1", squashB, 1, False)
            M("A2.0", emit_A, 2, 0)
            M("Z2.0", emit_Z, 2, 0)
            sp0 = M("W4_2.0", emit_W4, 2, 0)
            M("sqA2.0", squashA, 0, sp0, False, False)
            M("sqB2.0", squashB, 0, True)
            M("A2.1", emit_A, 2, 1)
            M("fin.0", emit_fin, 0)
            M("Z2.1", emit_Z, 2, 1)
            sp1 = M("W4_2.1", emit_W4, 2, 1)
            M("sqA2.1", squashA, 1, sp1, False, False)
            M("sqB2.1", squashB, 1, True)
            M("fin.1", emit_fin, 1)
            mark("out")
            nc.sync.dma_start(out_d[:], v16[:])
            mark("end")

    nc.compile()
    return nc


def _get_program(general_b):
    key = bool(general_b)
    if key not in _PROG_CACHE:
        _PROG_CACHE[key] = _build_program(key)
    return _PROG_CACHE[key]


def _prep_inputs(u_predict, b):
    general_b = bool(np.any(b != 0.0))
    consts = _build_consts()
    u16f = u_predict.astype(np.float16)
    # u16: [c, p=(bl,d), (j,h,i)]
    u6 = u16f.reshape(NCORES, H, BLOC, IC, OC, D)
    ut = np.ascontiguousarray(u6.transpose(0, 2, 5, 4, 1, 3))
    ut = ut.reshape(NCORES, 128, OC * H * NI)
    # ui: [c, p=i%128, (h,t,j,b,d)]
    u8 = u16f.reshape(NCORES, H, BLOC, NT, 128, OC, D)
    uit = np.ascontiguousarray(u8.transpose(0, 4, 1, 3, 5, 2, 6))
    uit = uit.reshape(NCORES, 128, H * NT * OC * 128)

    extra = {}
    if general_b:
        bm = b.astype(np.float64)
        e = np.exp(bm - bm.max(axis=1, keepdims=True))
        c0 = (e / e.sum(axis=1, keepdims=True))          # [IC, OC] f64
        c0i = np.ascontiguousarray(
            c0.reshape(NT, 128, OC).transpose(1, 0, 2)).reshape(
                128, NT * OC).astype(np.float16)
        eb = np.exp(bm - bm.max())                        # exp(b) global shift
        c0rep = np.ascontiguousarray(
            np.broadcast_to(eb.T.reshape(1, OC, NI),
                            (BLOC, OC, NI)).transpose(1, 0, 2)).reshape(
                OC * BLOC, NI).astype(np.float32)
        # M0 = exp(b)*x ; store exp(b) replicated over bl in (j,b)-rows
        import ml_dtypes
        extra = {"c0i": c0i,
                 "c0rep80": c0rep.astype(ml_dtypes.bfloat16)}

    in_maps = []
    for c in range(NCORES):
        m = {"u16": ut[c], "ui": uit[c]}
        m.update(consts)
        m.update(extra)
        in_maps.append(m)
    return in_maps, general_b


def _gather_output(results):
    out = np.empty((B, OC, D), np.float32)
    for c in range(NCORES):
        v = results[c]["vout"]                   # [(bl,d), (h,(j,b))]
        for h in range(H):
            vh = v[:, h * 80:(h + 1) * 80].reshape(BLOC, D, OC, BLOC)
            for bl in range(BLOC):
                out[c * BL + h * BLOC + bl] = vh[bl, :, :, bl].T
    return out


def kernel(u_predict, b=None, **kw):
    u_predict = np.asarray(u_predict, dtype=np.float32)
    if b is None:
        b = np.zeros((IC, OC), np.float32)
    b = np.asarray(b, dtype=np.float32)
    in_maps, general_b = _prep_inputs(u_predict, b)
    nc = _get_program(general_b)

    if os.environ.get("BASS_KERNEL_SIM"):
        from concourse.bass_interp import CoreSim
        sim = CoreSim(nc, trace=False)
        for name, arr in in_maps[0].items():
            sim.tensor(name)[:] = arr
        sim.simulate(check_with_hw=False)
        v0 = np.array(sim.tensor("vout"))
        out = np.zeros((B, OC, D), np.float32)
        for h in range(H):
            vh = v0[:, h * 80:(h + 1) * 80].reshape(BLOC, D, OC, BLOC)
            for bl in range(BLOC):
                out[h * BLOC + bl] = vh[bl, :, :, bl].T
        return out  # NOTE: only core 0 valid in sim mode

    from concourse import bass_utils
    trace = bool(os.environ.get("BASS_KERNEL_TRACE"))
    res = bass_utils.run_bass_kernel_spmd(
        nc, in_maps, core_ids=list(range(NCORES)), trace=trace)
    kernel.last_results = res
    return _gather_output(res.results)


# revision 5
# speedup vs baseline: 1.3080x; 1.1578x over previous
"""AgreementRouting (CapsNet dynamic routing) Trainium2 kernel, v2.

Data-parallel over batch B=128 across 8 cores (16 per core, split into
two halves h of 8). Per core, TWO SBUF copies of u (fp16):
  u16 [128=(b,d), (j,h,i)]  -- W1 moving operand (d-contraction)
  u_i [128=i%128, (h,t,j,(b,d))] -- W4 stationary tiles (i-contraction)

Routing state is kept in exp domain: M = exp(b0 + sum_t f_t a_t) (bf16,
unnormalized, range-safe), where a = s^T u on PE and the squash scale f
is applied inside the ACT exp as a per-partition scale. c = M / Z is
formed in i-partition layout (PE transposes + one DVE divide per tile)
and consumed by PE W4 matmuls with u stationary:
  s[(b,d), (j,b')] = sum_i u[i,(b,d)] c[b',i,j]   (diag b'==b valid)
The squash scale chain runs on [1,80] rows and returns to [80,1]
per-partition form via a PE transpose. No DMA inside the loop.
"""

import os
import sys

import numpy as np

for _p in ("/opt/trn_rl_repo", "/opt/trn_rl_repo/concourse"):
    if _p not in sys.path and os.path.isdir(_p):
        sys.path.insert(0, _p)

B, IC, OC, D = 128, 1152, 10, 16
NCORES = 8
BL = B // NCORES          # 16 local batch
H = 2                     # halves of local batch
BLOC = BL // H            # 8
NI = IC                   # 1152
NT = IC // 128            # 9 i-tiles
EPS = 1e-8
NITER = 3
CHUNKS = [(0, 512), (512, 1024), (1024, 1152)]

_PROG_CACHE = {}
BLOCKS = []


def _build_consts():
    mask = np.zeros((128, OC * BLOC), np.float16)
    for b in range(BLOC):
        for d in range(D):
            for j in range(OC):
                mask[b * D + d, j * BLOC + b] = 1.0
    eye80 = np.eye(80, dtype=np.float16)
    eyef32 = np.eye(8, dtype=np.float32)
    eye80f = np.eye(80, dtype=np.float32)
    onescol = np.ones((128, 1), np.float16)
    ones1 = np.ones((1, 128), np.float32)
    return dict(mask16=mask, eye80=eye80, eyef32=eyef32, eye80f=eye80f,
                onescol=onescol, ones1=ones1)


def _build_program(general_b):
    import concourse.bacc as bacc
    import concourse.mybir as mybir
    import concourse.tile as tile

    dt = mybir.dt
    AF = mybir.ActivationFunctionType
    ALU = mybir.AluOpType

    # Single shared ACT table (Exp+Ln+Copy+Identity) so the table-load
    # pass emits one load instead of thrashing per func.
    from concourse import hw_specs as _hws
    _orig_tabs = _hws.get_activation_tables
    _keep = "natural_log_exp_and_others"

    def _patched_tabs(arch, __orig=_orig_tabs, __keep=_keep):
        tabs = __orig(arch)
        return {n: (s if n == __keep else set()) for n, s in tabs.items()}

    bacc.get_activation_tables = _patched_tabs

    nc = bacc.Bacc("TRN2", target_bir_lowering=False, debug=False)

    # ---- DRAM I/O ----
    u16_d = nc.dram_tensor("u16", [128, OC * H * NI], dt.float16,
                           kind="ExternalInput").ap()
    ui_d = nc.dram_tensor("ui", [128, H * NT * OC * 128], dt.float16,
                          kind="ExternalInput").ap()
    mask_d = nc.dram_tensor("mask16", [128, OC * BLOC], dt.float16,
                            kind="ExternalInput").ap()
    eye80_d = nc.dram_tensor("eye80", [80, 80], dt.float16,
                             kind="ExternalInput").ap()
    eyef32_d = nc.dram_tensor("eyef32", [8, 8], dt.float32,
                              kind="ExternalInput").ap()
    eye80f_d = nc.dram_tensor("eye80f", [80, 80], dt.float32,
                              kind="ExternalInput").ap()
    onescol_d = nc.dram_tensor("onescol", [128, 1], dt.float16,
                               kind="ExternalInput").ap()
    ones1_d = nc.dram_tensor("ones1", [1, 128], dt.float32,
                             kind="ExternalInput").ap()
    if general_b:
        c0i_d = nc.dram_tensor("c0i", [128, NT * OC], dt.float16,
                               kind="ExternalInput").ap()
    out_d = nc.dram_tensor("vout", [128, H * OC * BLOC], dt.float32,
                           kind="ExternalOutput").ap()

    # ---- static SBUF ----
    def sb(name, shape, dtype):
        return nc.alloc_sbuf_tensor(name, list(shape), dtype).ap()

    u16 = sb("u16_sb", [128, OC * H * NI], dt.float16)
    ui = sb("ui_sb", [128, H * NT * OC * 128], dt.float16)
    mask = sb("mask_sb", [128, OC * BLOC], dt.float16)
    eye80 = sb("eye80_sb", [80, 80], dt.float16)
    eyef32 = sb("eyef32_sb", [8, 8], dt.float32)
    eye80f = sb("eye80f_sb", [80, 80], dt.float32)
    onescol = sb("onescol_sb", [128, 1], dt.float16)
    ones1 = sb("ones1_sb", [1, 128], dt.float32)
    dumw = sb("dumw_sb", [128, 80], dt.float16)
    s0sb = [sb(f"s0sb{h}", [128, OC], dt.float32) for h in range(H)]
    scr0 = sb("scr0_sb", [128, NI], dt.float16)
    xt = [sb(f"x{h}", [80, NI], dt.float16) for h in range(H)]
    zsum = [sb(f"zsum{h}", [128, NT * BLOC], dt.float32) for h in range(H)]
    zrec = [sb(f"zrec{h}", [128, NT * BLOC], dt.float32) for h in range(H)]
    ci = [sb(f"ci{h}", [128, NT * 80], dt.float16) for h in range(H)]
    s16 = [sb(f"s16_{h}", [128, 80], dt.float16) for h in range(H)]
    s16m = [sb(f"s16m{h}", [128, 80], dt.float16) for h in range(H)]
    ssq = [sb(f"ssq{h}", [128, 80], dt.float16) for h in range(H)]
    sfat_all = [sb(f"sfat_all{h}", [128, OC * 88], dt.float16)
                for h in range(H)]
    seps = [sb(f"seps{h}", [80, 1], dt.float32) for h in range(H)]
    lnx = [sb(f"lnx{h}", [80, 1], dt.float32) for h in range(H)]
    rr = [sb(f"rr{h}", [80, 1], dt.float32) for h in range(H)]
    t1 = [sb(f"t1_{h}", [80, 1], dt.float32) for h in range(H)]
    den = [sb(f"den{h}", [80, 1], dt.float32) for h in range(H)]
    rec = [sb(f"rec{h}", [80, 1], dt.float32) for h in range(H)]
    frow = [sb(f"frow{h}", [1, 80], dt.float32) for h in range(H)]
    f80sb = [sb(f"f80sb{h}", [80, 1], dt.float32) for h in range(H)]
    v16 = sb("v16_sb", [128, H * 80], dt.float32)
    if general_b:
        c0i = sb("c0i_sb", [128, NT * OC], dt.float16)

    ps_sA = nc.alloc_psum_tensor("ps_sA", [128, 512], dt.float32).ap()
    ps_sm0 = nc.alloc_psum_tensor("ps_sm0", [128, 512], dt.float32).ap()
    ps_mi = [nc.alloc_psum_tensor(f"ps_mi{h}", [128, 960],
                                  dt.float16).ap() for h in range(H)]

    def ps_s(h):
        return ps_sA[:, 80 * h:80 * h + 80]

    def ps_s0(h):
        return ps_sm0[:, 10 * h:10 * h + 10]

    def ps_sq(h):
        return ps_sm0[:80, 20 + h:21 + h]

    def ps_frow(h):
        return ps_sm0[:1, 24 + 80 * h:104 + 80 * h]

    def ps_fr(h):
        return ps_sm0[:, 182 + 80 * h:262 + 80 * h]

    def ps_mislot(h, t):
        return ps_mi[h][:, 80 * t:80 * t + 80]

    def sfatv(j, h):
        return sfat_all[h][:, j * 88:j * 88 + 80]

    def uview(j, h):
        off = (j * H + h) * NI
        return u16[:, off:off + NI]

    def uiview(h, t, j):
        off = ((h * NT + t) * OC + j) * 128
        return ui[:, off:off + 128]

    LN_C0 = float(np.log(1.0 / OC))

    def mark(label):
        BLOCKS.append((label, nc.next_id()))

    with tile.TileContext(nc) as tc:
        from contextlib import ExitStack
        with ExitStack() as ctx:
            psA = ctx.enter_context(
                tc.tile_pool(name="psA", bufs=4, space="PSUM"))

            # ---- loads ----
            # order: consts, u16-h0, u16-h1, ui-h0, ui-h1 so both s0
            # halves (DVE-accumulated from u16) start early; the W4s of
            # it0 gate on ui arrival instead.
            u16v_s = u16.rearrange("p (j h i) -> p j h i", j=OC, h=H)
            u16v_d = u16_d.rearrange("p (j h i) -> p j h i", j=OC, h=H)
            USPL = int(os.environ.get("K_USPL", "3"))
            ubnds = [-(-OC * k // USPL) for k in range(USPL + 1)]
            for j0, j1 in zip(ubnds, ubnds[1:]):
                nc.sync.dma_start(u16v_s[:, j0:j1, 0, :],
                                  u16v_d[:, j0:j1, 0, :])
            nc.sync.dma_start(mask[:], mask_d)
            nc.sync.dma_start(eye80[:], eye80_d)
            nc.sync.dma_start(eyef32[:], eyef32_d)
            nc.sync.dma_start(eye80f[:], eye80f_d)
            nc.sync.dma_start(onescol[:], onescol_d)
            nc.sync.dma_start(ones1[:], ones1_d)
            if general_b:
                nc.sync.dma_start(c0i[:], c0i_d)
            for j0, j1 in zip(ubnds, ubnds[1:]):
                nc.sync.dma_start(u16v_s[:, j0:j1, 1, :],
                                  u16v_d[:, j0:j1, 1, :])
            UIH = NT * OC * 128
            for h in range(H):
                NSPL = int(os.environ.get("K_UISPL", "5"))
                bnds = [1280 * (-(-NT * k // NSPL)) for k in range(NSPL + 1)]
                for f0, f1 in zip(bnds, bnds[1:]):
                    nc.sync.dma_start(
                        ui[:, h * UIH + f0:h * UIH + f1],
                        ui_d[:, h * UIH + f0:h * UIH + f1])

            nc.gpsimd.memset(dumw[:], 0.0)
            for h in range(H):
                if general_b:
                    nc.vector.tensor_copy(
                        out=ci[h].rearrange("p (t j b) -> p t j b",
                                            t=NT, j=OC),
                        in_=c0i.rearrange("p (t j) -> p t j", t=NT)
                        .unsqueeze(3).broadcast_to([128, NT, OC, BLOC]))
                else:
                    nc.gpsimd.memset(ci[h][:], 1.0 / OC)
            # touch the ACT table early so the first real Ln/Exp doesn't
            # pay the table load on the critical path
            nc.scalar.activation(f80sb[0][:1, 0:1], eyef32[:1, 0:1], AF.Exp)
            for h in range(H):
                nc.gpsimd.memset(sfat_all[h][:], 0.0)

            def squashA(h, s_src, bcast, build_sfat):
                """masked s + sfat blocks + ssq + sq matmul."""
                if bcast:
                    nc.vector.tensor_tensor(
                        s16m[h].rearrange("p (j b) -> p j b", j=OC),
                        s_src.unsqueeze(2).broadcast_to([128, OC, BLOC]),
                        mask.rearrange("p (j b) -> p j b", j=OC),
                        op=ALU.mult)
                else:
                    nc.vector.tensor_mul(s16m[h][:], s_src, mask[:])
                if build_sfat:
                    import bass_rust as _br
                    t = sfat_all[h]
                    outap = _br.AP(t.tensor, t.offset,
                                   [list(t.ap)[0], (96, OC), (1, BLOC)])
                    nc.vector.tensor_copy(
                        out=outap,
                        in_=s16m[h].rearrange("p (j b) -> p j b", j=OC))
                nc.vector.tensor_mul(ssq[h][:], s16m[h][:], s16m[h][:])
                sq_ps = ps_sq(h)
                nc.tensor.matmul(sq_ps, ssq[h][:], onescol[:],
                                 start=True, stop=True)

            def squashB(h, last):
                """squash scale chain, in [80,1] per-partition space."""
                sq_ps = ps_sq(h)
                nc.vector.tensor_scalar_add(seps[h][:], sq_ps, EPS)
                nc.scalar.activation(lnx[h][:], seps[h][:], AF.Ln)
                nc.scalar.activation(rr[h][:], lnx[h][:], AF.Exp, scale=0.5)
                nc.vector.tensor_scalar_add(t1[h][:], seps[h][:], 1.0)
                nc.vector.tensor_mul(den[h][:], t1[h][:], rr[h][:])
                nc.vector.reciprocal(rec[h][:], den[h][:])
                nc.vector.scalar_tensor_tensor(
                    out=f80sb[h][:], in0=t1[h][:], scalar=-1.0,
                    in1=rec[h][:], op0=ALU.add, op1=ALU.mult)
                if last:
                    fr_ps = ps_frow(h)
                    nc.tensor.matmul(fr_ps, f80sb[h][:], eye80f[:],
                                     start=True, stop=True)
                    nc.vector.tensor_copy(out=frow[h][:], in_=fr_ps)

            def squash(h, s_src, bcast, build_sfat, last):
                squashA(h, s_src, bcast, build_sfat)
                squashB(h, last)

            def emit_s0TS(h):
                """accumulate s0; returns the [128, OC] source ap."""
                if general_b:
                    s0_ps = ps_s0(h)
                    for t in range(NT):
                        for j in range(OC):
                            nc.tensor.matmul(
                                s0_ps[:, j:j + 1], uiview(h, t, j),
                                c0i[:, t * OC + j:t * OC + j + 1],
                                start=(t == 0 and j == 0),
                                stop=(t == NT - 1 and j == OC - 1),
                                skip_group_check=True)
                    return s0_ps
                # s0 = (1/OC) * sum_i u -- DVE 4x tensor_scalar+accum
                for j in range(OC):
                    nc.vector.tensor_scalar(
                        out=scr0[:], in0=uview(j, h), scalar1=1.0 / OC,
                        scalar2=0.0, op0=ALU.mult, op1=ALU.add,
                        accum_out=s0sb[h][:, j:j + 1])
                return s0sb[h][:]

            def emit_W1(it, h, cidx):
                c0, c1 = CHUNKS[cidx]
                a = psA.tile([80, 512], dt.float32, tag="a", name="a")
                for j in range(OC):
                    nc.tensor.matmul(a[:, :c1 - c0], sfatv(j, h),
                                     uview(j, h)[:, c0:c1],
                                     start=(j == 0), stop=(j == OC - 1))
                return a

            def emit_expM(it, h, cidx, a):
                c0, c1 = CHUNKS[cidx]
                nc.scalar.activation(
                    xt[h][:, c0:c1], a[:, :c1 - c0], AF.Exp,
                    scale=f80sb[h][:, 0:1])

            def emit_chunk(it, h, cidx):
                a = emit_W1(it, h, cidx)
                emit_expM(it, h, cidx, a)

            def emit_MT(h, t0, t1):
                for t in range(t0, t1):
                    mi = ps_mislot(h, t)
                    nc.tensor.matmul(mi, xt[h][:, t * 128:(t + 1) * 128],
                                     eye80[:], is_transpose=True,
                                     start=(t == 0), stop=(t == NT - 1),
                                     skip_group_check=True)

            def emit_red(h, t0, t1):
                nt = t1 - t0
                nc.vector.tensor_mul(
                    ci[h][:, t0 * 80:t1 * 80],
                    ci[h][:, t0 * 80:t1 * 80],
                    ps_mi[h][:, t0 * 80:t1 * 80])
                nc.vector.tensor_reduce(
                    out=zsum[h][:, t0 * BLOC:t1 * BLOC],
                    in_=ci[h][:, t0 * 80:t1 * 80].rearrange(
                        "p (t j b) -> p t b j", j=OC, t=nt),
                    axis=mybir.AxisListType.X, op=ALU.add)
                nc.vector.reciprocal(zrec[h][:, t0 * BLOC:t1 * BLOC],
                                     zsum[h][:, t0 * BLOC:t1 * BLOC])
                nc.vector.tensor_tensor(
                    ci[h][:, t0 * 80:t1 * 80].rearrange(
                        "p (t j b) -> p t j b", j=OC, t=nt),
                    ci[h][:, t0 * 80:t1 * 80].rearrange(
                        "p (t j b) -> p t j b", j=OC, t=nt),
                    zrec[h][:, t0 * BLOC:t1 * BLOC].rearrange(
                        "p (t b) -> p t b", t=nt)
                    .unsqueeze(2).broadcast_to([128, nt, OC, BLOC]),
                    op=ALU.mult)

            def emit_A(it, h):
                """W1 + exp + M-update, chunk-pipelined; the first 4
                M-transposes and their z/divide block are interleaved so
                W4 can start right after the last W1 chunk."""
                emit_chunk(it, h, 0)
                emit_chunk(it, h, 1)
                emit_MT(h, 0, 4)
                emit_red(h, 0, 4)
                emit_chunk(it, h, 2)

            def emit_Z(it, h):
                emit_MT(h, 4, NT)
                emit_red(h, 4, NT)

            def emit_W4(it, h):
                sp = ps_s(h)
                for t in range(NT):
                    for j in range(OC):
                        nc.tensor.matmul(
                            sp[:, j * BLOC:(j + 1) * BLOC],
                            uiview(h, t, j),
                            ci[h][:, t * 80 + j * BLOC:
                                  t * 80 + (j + 1) * BLOC],
                            start=(t == 0 and j == 0),
                            stop=(t == NT - 1 and j == OC - 1),
                            skip_group_check=True)
                return sp

            def emit_fin(h):
                frep = ps_fr(h)
                nc.tensor.matmul(frep, ones1[:], frow[h][:],
                                 start=True, stop=True)
                nc.vector.tensor_mul(v16[:, h * 80:(h + 1) * 80],
                                     s16m[h][:], frep)

            # ---- prewarm PE during the u16-h0 load tail ----
            ndum = int(os.environ.get("K_DUMMY", "6"))
            for k in range(ndum):
                dtile = psA.tile([80, 512], dt.float32, tag="a", name="dum")
                nc.tensor.matmul(dtile[:], dumw[:], u16[:, 0:512],
                                 start=True, stop=True)

            # ---- schedule ----
            # h0's data (u16-h0, ui-h0) lands first; h1's W4 of it0 gates
            # on the final ui-h1 DMA, so h0 runs one step ahead through
            # it0/it1 and h1 gets engine priority afterwards.
            def M(label, fn, *a):
                mark(label)
                return fn(*a)

            s0src = M("s0.0", emit_s0TS, 0)
            M("sqAs0.0", squashA, 0, s0src, True, True)
            a00 = M("W1c0.00", emit_W1, 0, 0, 0)
            M("sqBs0.0", squashB, 0, False)
            a01 = M("W1c1.00", emit_W1, 0, 0, 1)
            with tc.high_priority():
                s0src = M("s0.1", emit_s0TS, 1)
            M("exp0.00", emit_expM, 0, 0, 0, a00)
            M("exp1.00", emit_expM, 0, 0, 1, a01)
            M("MTa.00", emit_MT, 0, 0, 4)
            M("W1c2.00", emit_chunk, 0, 0, 2)
            M("sqAs0.1", squashA, 1, s0src, True, True)
            M("sqBs0.1", squashB, 1, False)
            M("reda.00", emit_red, 0, 0, 4)
            M("Z0.0", emit_Z, 0, 0)
            with tc.high_priority(offset=200):
                sp0 = M("W4_0.0", emit_W4, 0, 0)
            M("sqA0.0", squashA, 0, sp0, False, True)
            M("A0.1", emit_A, 0, 1)
            M("sqB0.0", squashB, 0, False)
            M("Z0.1", emit_Z, 0, 1)
            M("A1.0", emit_A, 1, 0)
            sp1 = M("W4_0.1", emit_W4, 0, 1)
            M("sqA0.1", squashA, 1, sp1, False, True)
            M("sqB0.1", squashB, 1, False)
            M("Z1.0", emit_Z, 1, 0)
            sp0 = M("W4_1.0", emit_W4, 1, 0)
            M("sqA1.0", squashA, 0, sp0, False, True)
            M("sqB1.0", squashB, 0, False)
            M("A1.1", emit_A, 1, 1)
            M("Z1.1", emit_Z, 1, 1)
            sp1 = M("W4_1.1", emit_W4, 1, 1)
            M("sqA1.1", squashA, 1, sp1, False, True)
            M("sqB1.1", squashB, 1, False)
            M("A2.0", emit_A, 2, 0)
            M("Z2.0", emit_Z, 2, 0)
            sp0 = M("W4_2.0", emit_W4, 2, 0)
            M("sqA2.0", squashA, 0, sp0, False, False)
            M("sqB2.0", squashB, 0, True)
            M("A2.1", emit_A, 2, 1)
            M("fin.0", emit_fin, 0)
            M("Z2.1", emit_Z, 2, 1)
            sp1 = M("W4_2.1", emit_W4, 2, 1)
            M("sqA2.1", squashA, 1, sp1, False, False)
            M("sqB2.1", squashB, 1, True)
            M("fin.1", emit_fin, 1)
            mark("out")
            nc.sync.dma_start(out_d[:], v16[:])
            mark("end")

    nc.compile()
    return nc


def _get_program(general_b):
    key = bool(general_b)
    if key not in _PROG_CACHE:
        _PROG_CACHE[key] = _build_program(key)
    return _PROG_CACHE[key]


def _prep_inputs(u_predict, b):
    general_b = bool(np.any(b != 0.0))
    consts = _build_consts()
    u16f = u_predict.astype(np.float16)
    # u16: [c, p=(bl,d), (j,h,i)]
    u6 = u16f.reshape(NCORES, H, BLOC, IC, OC, D)
    ut = np.ascontiguousarray(u6.transpose(0, 2, 5, 4, 1, 3))
    ut = ut.reshape(NCORES, 128, OC * H * NI)
    # ui: [c, p=i%128, (h,t,j,b,d)]
    u8 = u16f.reshape(NCORES, H, BLOC, NT, 128, OC, D)
    uit = np.ascontiguousarray(u8.transpose(0, 4, 1, 3, 5, 2, 6))
    uit = uit.reshape(NCORES, 128, H * NT * OC * 128)

    extra = {}
    if general_b:
        bm = b.astype(np.float64)
        e = np.exp(bm - bm.max(axis=1, keepdims=True))
        c0 = (e / e.sum(axis=1, keepdims=True))          # [IC, OC] f64
        c0i = np.ascontiguousarray(
            c0.reshape(NT, 128, OC).transpose(1, 0, 2)).reshape(
                128, NT * OC).astype(np.float16)
        eb = np.exp(bm - bm.max())                        # exp(b) global shift
        c0rep = np.ascontiguousarray(
            np.broadcast_to(eb.T.reshape(1, OC, NI),
                            (BLOC, OC, NI)).transpose(1, 0, 2)).reshape(
                OC * BLOC, NI).astype(np.float32)
        # M0 = exp(b)*x ; store exp(b) replicated over bl in (j,b)-rows
        import ml_dtypes
        extra = {"c0i": c0i,
                 "c0rep80": c0rep.astype(ml_dtypes.bfloat16)}

    in_maps = []
    for c in range(NCORES):
        m = {"u16": ut[c], "ui": uit[c]}
        m.update(consts)
        m.update(extra)
        in_maps.append(m)
    return in_maps, general_b


def _gather_output(results):
    out = np.empty((B, OC, D), np.float32)
    for c in range(NCORES):
        v = results[c]["vout"]                   # [(bl,d), (h,(j,b))]
        for h in range(H):
            vh = v[:, h * 80:(h + 1) * 80].reshape(BLOC, D, OC, BLOC)
            for bl in range(BLOC):
                out[c * BL + h * BLOC + bl] = vh[bl, :, :, bl].T
    return out


def kernel(u_predict, b=None, **kw):
    u_predict = np.asarray(u_predict, dtype=np.float32)
    if b is None:
        b = np.zeros((IC, OC), np.float32)
    b = np.asarray(b, dtype=np.float32)
    in_maps, general_b = _prep_inputs(u_predict, b)
    nc = _get_program(general_b)

    if os.environ.get("BASS_KERNEL_SIM"):
        from concourse.bass_interp import CoreSim
        sim = CoreSim(nc, trace=False)
        for name, arr in in_maps[0].items():
            sim.tensor(name)[:] = arr
        sim.simulate(check_with_hw=False)
        v0 = np.array(sim.tensor("vout"))
        out = np.zeros((B, OC, D), np.float32)
        for h in range(H):
            vh = v0[:, h * 80:(h + 1) * 80].reshape(BLOC, D, OC, BLOC)
            for bl in range(BLOC):
                out[h * BLOC + bl] = vh[bl, :, :, bl].T
        return out  # NOTE: only core 0 valid in sim mode

    from concourse import bass_utils
    trace = bool(os.environ.get("BASS_KERNEL_TRACE"))
    res = bass_utils.run_bass_kernel_spmd(
        nc, in_maps, core_ids=list(range(NCORES)), trace=trace)
    kernel.last_results = res
    return _gather_output(res.results)


# revision 6
# speedup vs baseline: 1.3359x; 1.0213x over previous
"""AgreementRouting (CapsNet dynamic routing) Trainium2 kernel, v2.

Data-parallel over batch B=128 across 8 cores (16 per core, split into
two halves h of 8). Per core, TWO SBUF copies of u (fp16):
  u16 [128=(b,d), (j,h,i)]  -- W1 moving operand (d-contraction)
  u_i [128=i%128, (h,t,j,(b,d))] -- W4 stationary tiles (i-contraction)

Routing state is kept in exp domain: M = exp(b0 + sum_t f_t a_t) (bf16,
unnormalized, range-safe), where a = s^T u on PE and the squash scale f
is applied inside the ACT exp as a per-partition scale. c = M / Z is
formed in i-partition layout (PE transposes + one DVE divide per tile)
and consumed by PE W4 matmuls with u stationary:
  s[(b,d), (j,b')] = sum_i u[i,(b,d)] c[b',i,j]   (diag b'==b valid)
The squash scale chain runs on [1,80] rows and returns to [80,1]
per-partition form via a PE transpose. No DMA inside the loop.
"""

import os
import sys

import numpy as np

for _p in ("/opt/trn_rl_repo", "/opt/trn_rl_repo/concourse"):
    if _p not in sys.path and os.path.isdir(_p):
        sys.path.insert(0, _p)

B, IC, OC, D = 128, 1152, 10, 16
NCORES = 8
BL = B // NCORES          # 16 local batch
H = 2                     # halves of local batch
BLOC = BL // H            # 8
NI = IC                   # 1152
NT = IC // 128            # 9 i-tiles
EPS = 1e-8
NITER = 3
CHUNKS = [(0, 512), (512, 1024), (1024, 1152)]

_PROG_CACHE = {}
BLOCKS = []


def _build_consts():
    mask = np.zeros((128, OC * BLOC), np.float16)
    for b in range(BLOC):
        for d in range(D):
            for j in range(OC):
                mask[b * D + d, j * BLOC + b] = 1.0
    eye80 = np.eye(80, dtype=np.float16)
    eyef32 = np.eye(8, dtype=np.float32)
    eye80f = np.eye(80, dtype=np.float32)
    onescol = np.ones((128, 1), np.float16)
    ones1 = np.ones((1, 128), np.float32)
    return dict(mask16=mask, eye80=eye80, eyef32=eyef32, eye80f=eye80f,
                onescol=onescol, ones1=ones1)


def _build_program(general_b):
    import concourse.bacc as bacc
    import concourse.mybir as mybir
    import concourse.tile as tile

    dt = mybir.dt
    AF = mybir.ActivationFunctionType
    ALU = mybir.AluOpType

    # Single shared ACT table (Exp+Ln+Copy+Identity) so the table-load
    # pass emits one load instead of thrashing per func.
    from concourse import hw_specs as _hws
    _orig_tabs = _hws.get_activation_tables
    _keep = "natural_log_exp_and_others"

    def _patched_tabs(arch, __orig=_orig_tabs, __keep=_keep):
        tabs = __orig(arch)
        return {n: (s if n == __keep else set()) for n, s in tabs.items()}

    bacc.get_activation_tables = _patched_tabs

    nc = bacc.Bacc("TRN2", target_bir_lowering=False, debug=False)

    # ---- DRAM I/O ----
    u16_d = nc.dram_tensor("u16", [128, OC * H * NI], dt.float16,
                           kind="ExternalInput").ap()
    ui_d = nc.dram_tensor("ui", [128, H * NT * OC * 128], dt.float16,
                          kind="ExternalInput").ap()
    mask_d = nc.dram_tensor("mask16", [128, OC * BLOC], dt.float16,
                            kind="ExternalInput").ap()
    eye80_d = nc.dram_tensor("eye80", [80, 80], dt.float16,
                             kind="ExternalInput").ap()
    eyef32_d = nc.dram_tensor("eyef32", [8, 8], dt.float32,
                              kind="ExternalInput").ap()
    eye80f_d = nc.dram_tensor("eye80f", [80, 80], dt.float32,
                              kind="ExternalInput").ap()
    onescol_d = nc.dram_tensor("onescol", [128, 1], dt.float16,
                               kind="ExternalInput").ap()
    ones1_d = nc.dram_tensor("ones1", [1, 128], dt.float32,
                             kind="ExternalInput").ap()
    if general_b:
        c0i_d = nc.dram_tensor("c0i", [128, NT * OC], dt.float16,
                               kind="ExternalInput").ap()
    out_d = nc.dram_tensor("vout", [128, H * OC * BLOC], dt.float32,
                           kind="ExternalOutput").ap()

    # ---- static SBUF ----
    def sb(name, shape, dtype):
        return nc.alloc_sbuf_tensor(name, list(shape), dtype).ap()

    u16 = sb("u16_sb", [128, OC * H * NI], dt.float16)
    ui = sb("ui_sb", [128, H * NT * OC * 128], dt.float16)
    mask = sb("mask_sb", [128, OC * BLOC], dt.float16)
    eye80 = sb("eye80_sb", [80, 80], dt.float16)
    eyef32 = sb("eyef32_sb", [8, 8], dt.float32)
    eye80f = sb("eye80f_sb", [80, 80], dt.float32)
    onescol = sb("onescol_sb", [128, 1], dt.float16)
    ones1 = sb("ones1_sb", [1, 128], dt.float32)
    dumw = sb("dumw_sb", [128, 80], dt.float16)
    s0sb = [sb(f"s0sb{h}", [128, OC], dt.float32) for h in range(H)]
    scr0 = sb("scr0_sb", [128, NI], dt.float16)
    scrA = sb("scrA_sb", [128, NI], dt.float16)
    xt = [sb(f"x{h}", [80, NI], dt.float16) for h in range(H)]
    zsum = [sb(f"zsum{h}", [128, NT * BLOC], dt.float32) for h in range(H)]
    zrec = [sb(f"zrec{h}", [128, NT * BLOC], dt.float32) for h in range(H)]
    zrec16 = [sb(f"zrec16{h}", [128, NT * BLOC], dt.float16)
              for h in range(H)]
    ci = [sb(f"ci{h}", [128, NT * 80], dt.float16) for h in range(H)]
    s16 = [sb(f"s16_{h}", [128, 80], dt.float16) for h in range(H)]
    s16m = [sb(f"s16m{h}", [128, 80], dt.float16) for h in range(H)]
    ssq = [sb(f"ssq{h}", [128, 80], dt.float16) for h in range(H)]
    sfat_all = [sb(f"sfat_all{h}", [128, OC * 88], dt.float16)
                for h in range(H)]
    seps = [sb(f"seps{h}", [80, 1], dt.float32) for h in range(H)]
    lnx = [sb(f"lnx{h}", [80, 1], dt.float32) for h in range(H)]
    rr = [sb(f"rr{h}", [80, 1], dt.float32) for h in range(H)]
    t1 = [sb(f"t1_{h}", [80, 1], dt.float32) for h in range(H)]
    den = [sb(f"den{h}", [80, 1], dt.float32) for h in range(H)]
    rec = [sb(f"rec{h}", [80, 1], dt.float32) for h in range(H)]
    frow = [sb(f"frow{h}", [1, 80], dt.float32) for h in range(H)]
    f80sb = [sb(f"f80sb{h}", [80, 1], dt.float32) for h in range(H)]
    v16 = sb("v16_sb", [128, H * 80], dt.float32)
    if general_b:
        c0i = sb("c0i_sb", [128, NT * OC], dt.float16)

    ps_sA = nc.alloc_psum_tensor("ps_sA", [128, 512], dt.float32).ap()
    ps_sm0 = nc.alloc_psum_tensor("ps_sm0", [128, 512], dt.float32).ap()
    ps_mi = [nc.alloc_psum_tensor(f"ps_mi{h}", [128, 960],
                                  dt.float16).ap() for h in range(H)]

    def ps_s(h):
        return ps_sA[:, 80 * h:80 * h + 80]

    def ps_s0(h):
        return ps_sm0[:, 10 * h:10 * h + 10]

    def ps_sq(h):
        return ps_sm0[:80, 20 + h:21 + h]

    def ps_frow(h):
        return ps_sm0[:1, 24 + 80 * h:104 + 80 * h]

    def ps_fr(h):
        return ps_sm0[:, 182 + 80 * h:262 + 80 * h]

    def ps_mislot(h, t):
        return ps_mi[h][:, 80 * t:80 * t + 80]

    def sfatv(j, h):
        return sfat_all[h][:, j * 88:j * 88 + 80]

    def uview(j, h):
        off = (j * H + h) * NI
        return u16[:, off:off + NI]

    def uiview(h, t, j):
        off = ((h * NT + t) * OC + j) * 128
        return ui[:, off:off + 128]

    LN_C0 = float(np.log(1.0 / OC))

    def mark(label):
        BLOCKS.append((label, nc.next_id()))

    with tile.TileContext(nc) as tc:
        from contextlib import ExitStack
        with ExitStack() as ctx:
            psA = ctx.enter_context(
                tc.tile_pool(name="psA", bufs=4, space="PSUM"))

            # ---- loads ----
            # order: consts, u16-h0, u16-h1, ui-h0, ui-h1 so both s0
            # halves (DVE-accumulated from u16) start early; the W4s of
            # it0 gate on ui arrival instead.
            u16v_s = u16.rearrange("p (j h i) -> p j h i", j=OC, h=H)
            u16v_d = u16_d.rearrange("p (j h i) -> p j h i", j=OC, h=H)
            USPL = int(os.environ.get("K_USPL", "3"))
            ubnds = [-(-OC * k // USPL) for k in range(USPL + 1)]
            for j0, j1 in zip(ubnds, ubnds[1:]):
                nc.sync.dma_start(u16v_s[:, j0:j1, 0, :],
                                  u16v_d[:, j0:j1, 0, :])
            nc.sync.dma_start(mask[:], mask_d)
            nc.sync.dma_start(eye80[:], eye80_d)
            nc.sync.dma_start(eyef32[:], eyef32_d)
            nc.sync.dma_start(eye80f[:], eye80f_d)
            nc.sync.dma_start(onescol[:], onescol_d)
            nc.sync.dma_start(ones1[:], ones1_d)
            if general_b:
                nc.sync.dma_start(c0i[:], c0i_d)
            for j0, j1 in zip(ubnds, ubnds[1:]):
                nc.sync.dma_start(u16v_s[:, j0:j1, 1, :],
                                  u16v_d[:, j0:j1, 1, :])
            UIH = NT * OC * 128
            for h in range(H):
                NSPL = int(os.environ.get("K_UISPL", "5"))
                bnds = [1280 * (-(-NT * k // NSPL)) for k in range(NSPL + 1)]
                for f0, f1 in zip(bnds, bnds[1:]):
                    nc.sync.dma_start(
                        ui[:, h * UIH + f0:h * UIH + f1],
                        ui_d[:, h * UIH + f0:h * UIH + f1])

            nc.gpsimd.memset(dumw[:], 0.0)
            for h in range(H):
                if general_b:
                    nc.vector.tensor_copy(
                        out=ci[h].rearrange("p (t j b) -> p t j b",
                                            t=NT, j=OC),
                        in_=c0i.rearrange("p (t j) -> p t j", t=NT)
                        .unsqueeze(3).broadcast_to([128, NT, OC, BLOC]))
                else:
                    nc.gpsimd.memset(ci[h][:], 1.0 / OC)
            # touch the ACT table early so the first real Ln/Exp doesn't
            # pay the table load on the critical path
            nc.scalar.activation(f80sb[0][:1, 0:1], eyef32[:1, 0:1], AF.Exp)
            for h in range(H):
                nc.gpsimd.memset(sfat_all[h][:], 0.0)

            def squashA(h, s_src, bcast, build_sfat):
                """masked s + sfat blocks + ssq + sq matmul."""
                if bcast:
                    nc.vector.tensor_tensor(
                        s16m[h].rearrange("p (j b) -> p j b", j=OC),
                        s_src.unsqueeze(2).broadcast_to([128, OC, BLOC]),
                        mask.rearrange("p (j b) -> p j b", j=OC),
                        op=ALU.mult)
                else:
                    nc.vector.tensor_mul(s16m[h][:], s_src, mask[:])
                if build_sfat:
                    import bass_rust as _br
                    t = sfat_all[h]
                    outap = _br.AP(t.tensor, t.offset,
                                   [list(t.ap)[0], (96, OC), (1, BLOC)])
                    nc.vector.tensor_copy(
                        out=outap,
                        in_=s16m[h].rearrange("p (j b) -> p j b", j=OC))
                nc.vector.tensor_mul(ssq[h][:], s16m[h][:], s16m[h][:])
                sq_ps = ps_sq(h)
                nc.tensor.matmul(sq_ps, ssq[h][:], onescol[:],
                                 start=True, stop=True)

            def squashB(h, last):
                """squash scale chain, in [80,1] per-partition space."""
                sq_ps = ps_sq(h)
                nc.vector.tensor_scalar_add(seps[h][:], sq_ps, EPS)
                nc.scalar.activation(lnx[h][:], seps[h][:], AF.Ln)
                nc.scalar.activation(rr[h][:], lnx[h][:], AF.Exp, scale=0.5)
                nc.vector.tensor_scalar_add(t1[h][:], seps[h][:], 1.0)
                nc.vector.tensor_mul(den[h][:], t1[h][:], rr[h][:])
                nc.vector.reciprocal(rec[h][:], den[h][:])
                nc.vector.scalar_tensor_tensor(
                    out=f80sb[h][:], in0=t1[h][:], scalar=-1.0,
                    in1=rec[h][:], op0=ALU.add, op1=ALU.mult)
                if last:
                    fr_ps = ps_frow(h)
                    nc.tensor.matmul(fr_ps, f80sb[h][:], eye80f[:],
                                     start=True, stop=True)
                    nc.vector.tensor_copy(out=frow[h][:], in_=fr_ps)

            def squash(h, s_src, bcast, build_sfat, last):
                squashA(h, s_src, bcast, build_sfat)
                squashB(h, last)

            def emit_s0TS(h):
                """accumulate s0; returns the [128, OC] source ap."""
                if general_b:
                    s0_ps = ps_s0(h)
                    for t in range(NT):
                        for j in range(OC):
                            nc.tensor.matmul(
                                s0_ps[:, j:j + 1], uiview(h, t, j),
                                c0i[:, t * OC + j:t * OC + j + 1],
                                start=(t == 0 and j == 0),
                                stop=(t == NT - 1 and j == OC - 1),
                                skip_group_check=True)
                    return s0_ps
                # s0 = (1/OC) * sum_i u -- DVE 4x tensor_scalar+accum;
                # for h1 the last two j's ride on the idle ACT engine so
                # the DVE tail doesn't gate the h1 ladder start.
                nact = int(os.environ.get("K_NACT1" if h else "K_NACT0", "2" if h else "0"))
                for j in range(OC - nact):
                    nc.vector.tensor_scalar(
                        out=scr0[:], in0=uview(j, h), scalar1=1.0 / OC,
                        scalar2=0.0, op0=ALU.mult, op1=ALU.add,
                        accum_out=s0sb[h][:, j:j + 1])
                for j in range(OC - nact, OC):
                    nc.scalar.activation(
                        scrA[:], uview(j, h), AF.Identity, scale=1.0 / OC,
                        accum_out=s0sb[h][:, j:j + 1])
                return s0sb[h][:]

            def emit_W1(it, h, cidx):
                c0, c1 = CHUNKS[cidx]
                a = psA.tile([80, 512], dt.float32, tag="a", name="a")
                for j in range(OC):
                    nc.tensor.matmul(a[:, :c1 - c0], sfatv(j, h),
                                     uview(j, h)[:, c0:c1],
                                     start=(j == 0), stop=(j == OC - 1))
                return a

            def emit_expM(it, h, cidx, a):
                c0, c1 = CHUNKS[cidx]
                nc.scalar.activation(
                    xt[h][:, c0:c1], a[:, :c1 - c0], AF.Exp,
                    scale=f80sb[h][:, 0:1])

            def emit_chunk(it, h, cidx):
                a = emit_W1(it, h, cidx)
                emit_expM(it, h, cidx, a)

            def emit_MT(h, t0, t1):
                for t in range(t0, t1):
                    mi = ps_mislot(h, t)
                    nc.tensor.matmul(mi, xt[h][:, t * 128:(t + 1) * 128],
                                     eye80[:], is_transpose=True,
                                     start=(t == 0), stop=(t == NT - 1),
                                     skip_group_check=True)

            def emit_red(h, t0, t1):
                nt = t1 - t0
                nc.vector.tensor_mul(
                    ci[h][:, t0 * 80:t1 * 80],
                    ci[h][:, t0 * 80:t1 * 80],
                    ps_mi[h][:, t0 * 80:t1 * 80])
                nc.vector.tensor_reduce(
                    out=zsum[h][:, t0 * BLOC:t1 * BLOC],
                    in_=ci[h][:, t0 * 80:t1 * 80].rearrange(
                        "p (t j b) -> p t b j", j=OC, t=nt),
                    axis=mybir.AxisListType.X, op=ALU.add)
                nc.vector.reciprocal(zrec[h][:, t0 * BLOC:t1 * BLOC],
                                     zsum[h][:, t0 * BLOC:t1 * BLOC])
                nc.vector.tensor_copy(
                    out=zrec16[h][:, t0 * BLOC:t1 * BLOC],
                    in_=zrec[h][:, t0 * BLOC:t1 * BLOC])
                nc.vector.tensor_tensor(
                    ci[h][:, t0 * 80:t1 * 80].rearrange(
                        "p (t j b) -> p t j b", j=OC, t=nt),
                    ci[h][:, t0 * 80:t1 * 80].rearrange(
                        "p (t j b) -> p t j b", j=OC, t=nt),
                    zrec16[h][:, t0 * BLOC:t1 * BLOC].rearrange(
                        "p (t b) -> p t b", t=nt)
                    .unsqueeze(2).broadcast_to([128, nt, OC, BLOC]),
                    op=ALU.mult)

            def emit_A(it, h):
                """W1 + exp + M-update, chunk-pipelined; the first 4
                M-transposes and their z/divide block are interleaved so
                W4 can start right after the last W1 chunk."""
                emit_chunk(it, h, 0)
                emit_chunk(it, h, 1)
                emit_MT(h, 0, 4)
                emit_red(h, 0, 4)
                emit_chunk(it, h, 2)

            def emit_Z(it, h):
                emit_MT(h, 4, NT)
                emit_red(h, 4, NT)

            def emit_W4(it, h):
                sp = ps_s(h)
                for t in range(NT):
                    for j in range(OC):
                        nc.tensor.matmul(
                            sp[:, j * BLOC:(j + 1) * BLOC],
                            uiview(h, t, j),
                            ci[h][:, t * 80 + j * BLOC:
                                  t * 80 + (j + 1) * BLOC],
                            start=(t == 0 and j == 0),
                            stop=(t == NT - 1 and j == OC - 1),
                            skip_group_check=True)
                return sp

            def emit_fin(h):
                frep = ps_fr(h)
                nc.tensor.matmul(frep, ones1[:], frow[h][:],
                                 start=True, stop=True)
                nc.vector.tensor_mul(v16[:, h * 80:(h + 1) * 80],
                                     s16m[h][:], frep)

            # ---- prewarm PE during the u16-h0 load tail ----
            ndum = int(os.environ.get("K_DUMMY", "6"))
            for k in range(ndum):
                dtile = psA.tile([80, 512], dt.float32, tag="a", name="dum")
                nc.tensor.matmul(dtile[:], dumw[:], u16[:, 0:512],
                                 start=True, stop=True)

            # ---- schedule ----
            # h0's data (u16-h0, ui-h0) lands first; h1's W4 of it0 gates
            # on the final ui-h1 DMA, so h0 runs one step ahead through
            # it0/it1 and h1 gets engine priority afterwards.
            def M(label, fn, *a):
                mark(label)
                return fn(*a)

            s0src = M("s0.0", emit_s0TS, 0)
            M("sqAs0.0", squashA, 0, s0src, True, True)
            a00 = M("W1c0.00", emit_W1, 0, 0, 0)
            M("sqBs0.0", squashB, 0, False)
            a01 = M("W1c1.00", emit_W1, 0, 0, 1)
            with tc.high_priority():
                s0src = M("s0.1", emit_s0TS, 1)
            M("exp0.00", emit_expM, 0, 0, 0, a00)
            M("exp1.00", emit_expM, 0, 0, 1, a01)
            M("MTa.00", emit_MT, 0, 0, 4)
            M("W1c2.00", emit_chunk, 0, 0, 2)
            M("sqAs0.1", squashA, 1, s0src, True, True)
            M("sqBs0.1", squashB, 1, False)
            M("reda.00", emit_red, 0, 0, 4)
            M("Z0.0", emit_Z, 0, 0)
            with tc.high_priority(offset=200):
                sp0 = M("W4_0.0", emit_W4, 0, 0)
            M("sqA0.0", squashA, 0, sp0, False, True)
            M("A0.1", emit_A, 0, 1)
            M("sqB0.0", squashB, 0, False)
            M("Z0.1", emit_Z, 0, 1)
            M("A1.0", emit_A, 1, 0)
            sp1 = M("W4_0.1", emit_W4, 0, 1)
            M("sqA0.1", squashA, 1, sp1, False, True)
            M("sqB0.1", squashB, 1, False)
            M("Z1.0", emit_Z, 1, 0)
            sp0 = M("W4_1.0", emit_W4, 1, 0)
            M("sqA1.0", squashA, 0, sp0, False, True)
            M("sqB1.0", squashB, 0, False)
            M("A1.1", emit_A, 1, 1)
            M("Z1.1", emit_Z, 1, 1)
            sp1 = M("W4_1.1", emit_W4, 1, 1)
            M("sqA1.1", squashA, 1, sp1, False, True)
            M("sqB1.1", squashB, 1, False)
            M("A2.0", emit_A, 2, 0)
            M("Z2.0", emit_Z, 2, 0)
            sp0 = M("W4_2.0", emit_W4, 2, 0)
            M("sqA2.0", squashA, 0, sp0, False, False)
            M("sqB2.0", squashB, 0, True)
            M("A2.1", emit_A, 2, 1)
            M("fin.0", emit_fin, 0)
            M("Z2.1", emit_Z, 2, 1)
            sp1 = M("W4_2.1", emit_W4, 2, 1)
            M("sqA2.1", squashA, 1, sp1, False, False)
            M("sqB2.1", squashB, 1, True)
            mark("out0")
            nc.sync.dma_start(out_d[:, :80], v16[:, :80])
            M("fin.1", emit_fin, 1)
            mark("out")
            nc.sync.dma_start(out_d[:, 80:], v16[:, 80:])
            mark("end")

    nc.compile()
    return nc


def _get_program(general_b):
    key = bool(general_b)
    if key not in _PROG_CACHE:
        _PROG_CACHE[key] = _build_program(key)
    return _PROG_CACHE[key]


def _prep_inputs(u_predict, b):
    general_b = bool(np.any(b != 0.0))
    consts = _build_consts()
    u16f = u_predict.astype(np.float16)
    # u16: [c, p=(bl,d), (j,h,i)]
    u6 = u16f.reshape(NCORES, H, BLOC, IC, OC, D)
    ut = np.ascontiguousarray(u6.transpose(0, 2, 5, 4, 1, 3))
    ut = ut.reshape(NCORES, 128, OC * H * NI)
    # ui: [c, p=i%128, (h,t,j,b,d)]
    u8 = u16f.reshape(NCORES, H, BLOC, NT, 128, OC, D)
    uit = np.ascontiguousarray(u8.transpose(0, 4, 1, 3, 5, 2, 6))
    uit = uit.reshape(NCORES, 128, H * NT * OC * 128)

    extra = {}
    if general_b:
        bm = b.astype(np.float64)
        e = np.exp(bm - bm.max(axis=1, keepdims=True))
        c0 = (e / e.sum(axis=1, keepdims=True))          # [IC, OC] f64
        c0i = np.ascontiguousarray(
            c0.reshape(NT, 128, OC).transpose(1, 0, 2)).reshape(
                128, NT * OC).astype(np.float16)
        extra = {"c0i": c0i}

    in_maps = []
    for c in range(NCORES):
        m = {"u16": ut[c], "ui": uit[c]}
        m.update(consts)
        m.update(extra)
        in_maps.append(m)
    return in_maps, general_b


def _gather_output(results):
    out = np.empty((B, OC, D), np.float32)
    for c in range(NCORES):
        v = results[c]["vout"]                   # [(bl,d), (h,(j,b))]
        for h in range(H):
            vh = v[:, h * 80:(h + 1) * 80].reshape(BLOC, D, OC, BLOC)
            for bl in range(BLOC):
                out[c * BL + h * BLOC + bl] = vh[bl, :, :, bl].T
    return out


def kernel(u_predict, b=None, **kw):
    u_predict = np.asarray(u_predict, dtype=np.float32)
    if b is None:
        b = np.zeros((IC, OC), np.float32)
    b = np.asarray(b, dtype=np.float32)
    in_maps, general_b = _prep_inputs(u_predict, b)
    nc = _get_program(general_b)

    if os.environ.get("BASS_KERNEL_SIM"):
        from concourse.bass_interp import CoreSim
        sim = CoreSim(nc, trace=False)
        for name, arr in in_maps[0].items():
            sim.tensor(name)[:] = arr
        sim.simulate(check_with_hw=False)
        v0 = np.array(sim.tensor("vout"))
        out = np.zeros((B, OC, D), np.float32)
        for h in range(H):
            vh = v0[:, h * 80:(h + 1) * 80].reshape(BLOC, D, OC, BLOC)
            for bl in range(BLOC):
                out[h * BLOC + bl] = vh[bl, :, :, bl].T
        return out  # NOTE: only core 0 valid in sim mode

    from concourse import bass_utils
    trace = bool(os.environ.get("BASS_KERNEL_TRACE"))
    res = bass_utils.run_bass_kernel_spmd(
        nc, in_maps, core_ids=list(range(NCORES)), trace=trace)
    kernel.last_results = res
    return _gather_output(res.results)
